# revision 51
# baseline (speedup 1.0000x reference)
"""Distributed Bass attention kernel for 8 TRN2 NeuronCores.

Problem: single-head causal attention, B=4, S=2048, d_model=1024, d_head=64.
  q = x@WQ.T+bq; k = x@WK.T+bk; v = x@WV.T+bv (v is d_model wide)
  out = softmax(causal(q@k.T)) @ v

Sharding: core = 2*b + half. Each core computes batch b, output channels
[half*512, (half+1)*512). Q/K/scores/softmax are duplicated within a batch
pair (cheap); V projection and attn@V are channel-split. No collectives.

Layout tricks:
  - x fed pre-transposed and bf16 (xT [d, S]); wv chunks are interleaved
    with xt chunks in the input stream so the PE can run Q/K projection
    (4 chunk-outer PSUM accumulators) plus the first 4 V tiles (4 more
    accumulators) against each chunk as it lands - the PE never starves
    during the input stream.
  - Q/K projections are evacuated to an f32r qkT tile with the bias added
    as a per-partition tensor_scalar; scores use a single fp32r matmul
    (K=64) per key tile - fp32r runs at bf16 speed for free dims >= 256
    and keeps both q and k at high precision.
  - scores computed transposed [keys, queries] so the exp'd P tiles are
    directly the stationary operand of attn@V - no transposes anywhere.
  - diagonal score tiles shrink their query range (512/384/256/256 per
    block); only the [128,128] triangle needs masking.
  - softmax without max-subtraction (|logits| <= ~50 => exp fits fp32).
    Per-query key-sums accumulate as two tile-sum chains (DVE + GpSimd);
    the 128-way cross-partition reduction is a 1-row ones-matmul per
    query tile, reciprocal on DVE, and the divide is folded into the
    attn@V PSUM->SBUF evac (ACT Copy with per-partition scale). Output
    leaves the chip normalized, in bf16.
  - attention blocks processed in reverse size order and software-
    pipelined: block j's attn@V matmuls interleave into block j-1's
    scores emission so the in-order PE queue never stalls; dummy warm-up
    matmuls cover the initial DMA window (and release the HAM clock gate).
"""

import sys

if "/opt/trn_rl_repo" not in sys.path:
    sys.path.insert(0, "/opt/trn_rl_repo")

import numpy as np

from concourse import bacc, tile, mybir
import concourse.bass as bass
from concourse.bass_utils import run_bass_kernel_spmd

B, S, D, HD = 4, 2048, 1024, 64
N_CORES = 8
CPC = 512  # output channels per core
NCHUNK = 8  # d_model / 128

f32 = mybir.dt.float32
f32r = mybir.dt.float32r
bf16 = mybir.dt.bfloat16
AF = mybir.ActivationFunctionType
ALU = mybir.AluOpType

# width of the scores matmul for diagonal-group tile m (block-relative);
# fp32r needs >= 256 moving rows for full speed so m=3 stays at 256 with
# only its upper half (the triangle) valid.
DIAG_W = [512, 384, 256, 256]

_cache = {}


def _scores_width(j, i):
    m = i - 4 * j
    return 512 if m < 0 else DIAG_W[m]


def _build():
    nc = bacc.Bacc("TRN2", target_bir_lowering=False, debug=False, num_devices=N_CORES)

    xT = nc.dram_tensor("xT", [NCHUNK, 128, S], bf16, kind="ExternalInput")
    wqkT = nc.dram_tensor("wqkT", [128, NCHUNK, 128], bf16, kind="ExternalInput")
    bqkc = nc.dram_tensor("bqkc", [128, 1], f32, kind="ExternalInput")
    wvT2 = nc.dram_tensor("wvT2", [128, NCHUNK, CPC], bf16, kind="ExternalInput")
    trim = nc.dram_tensor("trim", [128, 128], bf16, kind="ExternalInput")
    out = nc.dram_tensor("out", [128, 16, CPC], bf16, kind="ExternalOutput")

    with tile.TileContext(nc) as tc:
        with (
            tc.tile_pool(name="big", bufs=1) as big,
            tc.tile_pool(name="ppool", bufs=30) as ppool,
            tc.tile_pool(name="opool", bufs=8) as opool,
            tc.tile_pool(name="spool", bufs=3) as spool,
            tc.tile_pool(name="rpool", bufs=8) as rpool,
            tc.tile_pool(name="ps_a", bufs=4, space=bass.MemorySpace.PSUM) as ps_a,
            tc.tile_pool(name="ps_b", bufs=4, space=bass.MemorySpace.PSUM) as ps_b,
        ):
            # persistent SBUF tiles
            xt = big.tile([128, NCHUNK, S], bf16, tag="xt")  # 32KB/p
            wqk = big.tile([128, NCHUNK, 128], bf16, tag="wqk")  # 2KB/p
            wv = big.tile([128, NCHUNK, CPC], bf16, tag="wv")  # 8KB/p
            bqk_sb = big.tile([128, 1], f32, tag="bqk")
            tri_sb = big.tile([128, 128], bf16, tag="tri")
            qke = big.tile([128, 4, 512], f32r, tag="qke")  # q rows 0-63, k 64-127
            kT = big.tile([64, 4, 512], f32r, tag="kT")  # k shifted to base 0
            v_sb = big.tile([128, 16, CPC], bf16, tag="v")  # 16KB/p
            ones_w = big.tile([128, 256], bf16, tag="ones_w")
            ones_c = big.tile([128, 1], f32, tag="ones_c")

            # input DMAs. SP.SEQ dispatch costs ~650ns per dma_start, so the
            # stream uses few, large transfers in the order the PE consumes
            # them; tiny transfers ride the otherwise-idle Pool (SWDGE)
            # queue so they never delay an xt chunk.
            nc.gpsimd.memset(ones_w[:, :], 1.0)
            nc.gpsimd.dma_start(out=bqk_sb[:, :], in_=bqkc[:, :])
            nc.gpsimd.dma_start(out=tri_sb[:, :], in_=trim[:, :])
            nc.sync.dma_start(out=wqk[:, :, :], in_=wqkT[:, :, :])
            # stream column-halves: h0 (x cols 0-1024, feeds qk blocks 0-1 +
            # v tiles 0-7) first with wv pairs slotted in, h1 afterwards
            # (feeds qk blocks 2-3 + v tiles 8-15). Keeps every phase
            # PE-bound instead of waiting on the full 4MB of x.
            H = S // 2
            for c in range(NCHUNK):
                nc.sync.dma_start(out=xt[:, c, 0:H], in_=xT[c, :, 0:H])
                if c % 2 == 0:
                    nc.sync.dma_start(
                        out=wv[:, c : c + 2, :], in_=wvT2[:, c : c + 2, :]
                    )
            for c in range(NCHUNK):
                nc.sync.dma_start(out=xt[:, c, H:S], in_=xT[c, :, H:S])
            nc.gpsimd.memset(ones_c[:, :], 1.0)

            # PE warmup: dummy matmuls on the ones tile while input DMA
            # streams, so the HAM clock-gate is released before real work.
            # Chained into out[0] (overwritten later) so DCE keeps them.
            warm_ps = ps_b.tile([128, 512], f32, tag="B", name="warm_ps")
            NWARM = 10
            for w in range(NWARM):
                nc.tensor.matmul(
                    warm_ps[:, 0:256],
                    ones_w[:, 0:128],
                    ones_w[:, 0:256],
                    start=(w == 0),
                    stop=(w == NWARM - 1),
                )
            warm_sb = opool.tile([128, CPC], bf16, tag="osb", name="warm_sb")
            nc.scalar.copy(warm_sb[:, 0:256], warm_ps[:, 0:256])
            nc.sync.dma_start(out=out[:, 0, 0:256], in_=warm_sb[:, 0:256])

            # ---- streaming phase 1 (h0): QK blocks 0-1 + V tiles 0-3,
            # chunk-outer so each h0 half-chunk is consumed as it lands ----
            qk_ps = {}
            for j in (0, 1):
                qk_ps[j] = ps_a.tile([128, 512], f32, tag="A", name=f"qkps{j}")
            vp_ps = [
                ps_b.tile([128, CPC], f32, tag="B", name=f"vps{t}") for t in range(4)
            ]
            for c in range(NCHUNK):
                st, sp = (c == 0), (c == NCHUNK - 1)
                for j in (0, 1):
                    nc.tensor.matmul(
                        qk_ps[j][:, :],
                        wqk[:, c, :],
                        xt[:, c, 512 * j : 512 * (j + 1)],
                        start=st,
                        stop=sp,
                    )
                for t in range(4):
                    nc.tensor.matmul(
                        vp_ps[t][:, :],
                        xt[:, c, 128 * t : 128 * (t + 1)],
                        wv[:, c, :],
                        start=st,
                        stop=sp,
                    )

            def qk_evac(j):
                # qke = qk_ps + bias (per-partition) as f32r, then a
                # partition-shifting SBUF DMA brings the k half down to
                # kT's base partition 0 so scores operands lane-align.
                nc.vector.tensor_scalar_add(qke[:, j, :], qk_ps[j][:, :], bqk_sb[:, :])
                nc.scalar.dma_start(out=kT[:, j, :], in_=qke[64:128, j, :])

            def v_evac(t, ps):
                if t % 2 == 0:
                    nc.scalar.copy(v_sb[:, t, :], ps[:, :])
                else:
                    nc.vector.tensor_copy(v_sb[:, t, :], ps[:, :])

            for j in (0, 1):
                qk_evac(j)
            for t in range(4):
                v_evac(t, vp_ps[t])

            # ---- streaming phase 2 (h1): QK blocks 3,2 chunk-outer over
            # the arriving h1 half-chunks; V tiles 4-7 (h0-resident data)
            # fill the PE between chunks ----
            for j in (3, 2):
                qk_ps[j] = ps_a.tile([128, 512], f32, tag="A", name=f"qkps{j}")
            vp47 = {}
            for c in range(NCHUNK):
                st, sp = (c == 0), (c == NCHUNK - 1)
                for j in (3, 2):
                    nc.tensor.matmul(
                        qk_ps[j][:, :],
                        wqk[:, c, :],
                        xt[:, c, 512 * j : 512 * (j + 1)],
                        start=st,
                        stop=sp,
                    )
                if c % 2 == 1:
                    t = 4 + c // 2
                    vp47[t] = ps_b.tile([128, CPC], f32, tag="B", name=f"vps{t}")
                    for cc in range(NCHUNK):
                        nc.tensor.matmul(
                            vp47[t][:, :],
                            xt[:, cc, 128 * t : 128 * (t + 1)],
                            wv[:, cc, :],
                            start=(cc == 0),
                            stop=(cc == NCHUNK - 1),
                        )
            for j in (3, 2):
                qk_evac(j)
            for t in range(4, 8):
                v_evac(t, vp47[t])

            # ---- attention machinery ----
            def emit_scores(j, i, Ssum, P):
                """scores^T tile [keys(i), queries(block j)] -> exp -> chain."""
                m = i - 4 * j
                W = _scores_width(j, i)
                sc = ps_a.tile([128, 512], f32, tag="A", name=f"sc{j}_{i}")
                nc.tensor.matmul(
                    sc[:, 0:W],
                    kT[:, i // 4, 128 * (i % 4) : 128 * (i % 4) + 128],
                    qke[0:64, j, 512 - W : 512],
                    start=True,
                    stop=True,
                )
                p = ppool.tile([128, 512], bf16, tag="p", name=f"p{j}_{i}")
                nc.scalar.activation(p[:, 0:W], sc[:, 0:W], AF.Exp)
                cs = 0  # start of the valid (chained) columns within p
                if m >= 0:
                    if m < 3:
                        nc.vector.tensor_tensor(
                            p[:, 0:128], p[:, 0:128], tri_sb[:, :], ALU.mult
                        )
                    else:
                        cs = 128
                        nc.vector.tensor_tensor(
                            p[:, 128:256], p[:, 128:256], tri_sb[:, :], ALU.mult
                        )
                ch = i % 2
                # block 0's chains stay on DVE: the Pool engine must be free
                # at the end for the final block's SWDGE store descgens
                eng = nc.vector if (ch == 0 or j == 0) else nc.gpsimd
                Sc = Ssum[ch]
                lo = 512 - W + cs
                if i < 2:
                    if lo > 0:
                        eng.memset(Sc[:, 0:lo], 0.0)
                    eng.tensor_copy(Sc[:, lo:512], p[:, cs:W])
                else:
                    eng.tensor_tensor(Sc[:, lo:512], Sc[:, lo:512], p[:, cs:W], ALU.add)
                P.append((p, W))

            def attnv_ops(j, reverse=False):
                # the final block leads with its smallest tile so the first
                # store chain dispatches early, then descending sizes so the
                # trailing chains pipeline behind the remaining matmuls
                ops = []
                first = True
                for tq in ([0, 3, 2, 1] if reverse else range(4)):
                    t = 4 * j + tq
                    ops.append(("alloc", t))
                    for i in range(t + 1):
                        ops.append(("mm", t, i))
                    if first:
                        # rowsum -> reciprocal for the whole block; chains
                        # are surely done by the end of the first tile's mms
                        ops.append(("rs", j))
                        first = False
                    ops.append(("evac", t))
                return ops

            def emit_attnv_op(op, j, P, state, Ssum, rcps):
                if op[0] == "alloc":
                    t = op[1]
                    state[t] = ps_b.tile([128, CPC], f32, tag="B", name=f"ops{t}")
                elif op[0] == "mm":
                    _, t, i = op
                    p, W = P[i]
                    co = 128 * (t % 4) - (512 - W)
                    nc.tensor.matmul(
                        state[t][:, :],
                        p[:, co : co + 128],
                        v_sb[:, i, :],
                        start=(i == 0),
                        stop=(i == t),
                    )
                elif op[0] == "rs":
                    # fold chain 0 into chain 1, then one 1-row ones-matmul
                    # per query tile gives the 128-way key sum transposed to
                    # [query, 1]; reciprocal on DVE feeds the evac scale.
                    # All four sums land in one PSUM tile (one tag-A slot,
                    # sequential groups into disjoint columns).
                    nc.vector.tensor_tensor(
                        Ssum[1][:, :], Ssum[1][:, :], Ssum[0][:, :], ALU.add
                    )
                    rs = ps_a.tile([128, 4], f32, tag="A", name=f"rs{j}")
                    for tq in range(4):
                        nc.tensor.matmul(
                            rs[:, tq : tq + 1],
                            Ssum[1][:, 128 * tq : 128 * (tq + 1)],
                            ones_c[:, :],
                            start=True,
                            stop=True,
                        )
                    rcp4 = rpool.tile([128, 4], f32, tag="rcp", name=f"rcp{j}")
                    nc.vector.reciprocal(rcp4, rs[:, 0:4])
                    for tq in range(4):
                        rcps[4 * j + tq] = rcp4[:, tq : tq + 1]
                else:
                    t = op[1]
                    dst = opool.tile([128, CPC], bf16, tag="osb", name=f"osb{t}")
                    # alternate evac engine and DMA queue so back-to-back
                    # evac+store chains dispatch two at a time
                    if t % 2 == 0:
                        nc.scalar.activation(
                            dst, state[t][:, :], AF.Copy, scale=rcps[t]
                        )
                        nc.sync.dma_start(out=out[:, t, :], in_=dst)
                    else:
                        nc.vector.tensor_scalar_mul(dst, state[t][:, :], rcps[t])
                        # final block's odd tiles store via the idle Pool
                        # SWDGE - their descgens run parallel to the HWDGE
                        # chain draining the even tiles at the very end
                        q = nc.gpsimd if t < 4 else nc.scalar
                        q.dma_start(out=out[:, t, :], in_=dst)

            rcps = {}

            # ---- V tiles 8-15, with block 3's scores interleaved so its
            # exp chain (ACT) finishes before attn@V needs P ----
            Ssum3 = [
                spool.tile([128, 512], f32, tag=f"S{c}", name=f"S3_{c}")
                for c in range(2)
            ]
            P3 = []
            nxt = 0
            for t in range(8, 16):
                v_ps = ps_b.tile([128, CPC], f32, tag="B", name=f"vps{t}")
                for c in range(NCHUNK):
                    nc.tensor.matmul(
                        v_ps[:, :],
                        xt[:, c, 128 * t : 128 * (t + 1)],
                        wv[:, c, :],
                        start=(c == 0),
                        stop=(c == NCHUNK - 1),
                    )
                v_evac(t, v_ps)
                for _ in range(2):
                    emit_scores(3, nxt, Ssum3, P3)
                    nxt += 1
            while nxt < 16:
                emit_scores(3, nxt, Ssum3, P3)
                nxt += 1

            prev = (3, P3, Ssum3)  # block 3 scored during vproj; attn@V pending
            for j in [2, 1]:
                pj, pP, pS = prev
                av = attnv_ops(pj)
                av_state = {}
                n = 4 * j + 4
                Ssum = [
                    spool.tile([128, 512], f32, tag=f"S{c}", name=f"S{j}_{c}")
                    for c in range(2)
                ]
                P = []
                front = min(3, n)
                k_av = 0
                for idx in range(n):
                    emit_scores(j, idx, Ssum, P)
                    if idx >= front - 1:
                        want = (idx + 1 - front + 1) * len(av) / max(1, n - front + 1)
                        while k_av < len(av) and k_av < want:
                            emit_attnv_op(av[k_av], pj, pP, av_state, pS, rcps)
                            k_av += 1
                while k_av < len(av):
                    emit_attnv_op(av[k_av], pj, pP, av_state, pS, rcps)
                    k_av += 1
                prev = (j, P, Ssum)

            # ---- phase: scores(0) interleaved into attn@V(1) ----
            pj, pP, pS = prev  # block 1
            av = attnv_ops(1)
            av_state = {}
            Ssum0 = [
                spool.tile([128, 512], f32, tag=f"S{c}", name=f"S0_{c}")
                for c in range(2)
            ]
            P0 = []
            k_av = 0
            for idx in range(4):
                emit_scores(0, idx, Ssum0, P0)
                if idx >= 2:
                    want = (idx - 1) * len(av) / 2
                    while k_av < len(av) and k_av < want:
                        emit_attnv_op(av[k_av], 1, pP, av_state, pS, rcps)
                        k_av += 1
            while k_av < len(av):
                emit_attnv_op(av[k_av], 1, pP, av_state, pS, rcps)
                k_av += 1

            # ---- final phase: attn@V(0), smallest tile first ----
            st0 = {}
            for op in attnv_ops(0, reverse=True):
                emit_attnv_op(op, 0, P0, st0, Ssum0, rcps)

    nc.compile()
    return nc


def _get_nc():
    if "nc" not in _cache:
        _cache["nc"] = _build()
    return _cache["nc"]


def _prep_in_maps(x, WQ_w, WQ_b, WK_w, WK_b, WV_w, WV_b):
    bf = mybir.dt.np(bf16)
    wqk = np.concatenate([WQ_w, WK_w], axis=0)  # [128, D]
    wqkT = np.ascontiguousarray(
        wqk.T.reshape(NCHUNK, 128, 128).transpose(1, 0, 2)
    ).astype(bf)
    bqkc = np.concatenate([WQ_b, WK_b]).reshape(128, 1).astype(np.float32)

    kk = np.arange(128)[:, None]
    qq = np.arange(128)[None, :]
    trim = (kk <= qq).astype(bf)

    in_maps = []
    for core in range(N_CORES):
        b, half = core // 2, core % 2
        xTb = np.ascontiguousarray(x[b].T).reshape(NCHUNK, 128, S)
        wv_sl = WV_w[half * CPC : (half + 1) * CPC]  # [CPC, D]
        wvT2 = np.ascontiguousarray(
            wv_sl.T.reshape(NCHUNK, 128, CPC).transpose(1, 0, 2)
        )
        in_maps.append(
            {
                "xT": xTb.astype(bf),
                "wqkT": wqkT,
                "bqkc": bqkc,
                "wvT2": wvT2.astype(bf),
                "trim": trim,
            }
        )
    return in_maps


def _run(in_maps, trace=False, **kw):
    nc = _get_nc()
    return run_bass_kernel_spmd(
        nc, in_maps, core_ids=list(range(N_CORES)), trace=trace, **kw
    )


def kernel(x, WQ_w, WQ_b, WK_w, WK_b, WV_w, WV_b):
    x = np.asarray(x, dtype=np.float32)
    WV_b = np.asarray(WV_b, np.float32)
    in_maps = _prep_in_maps(
        x,
        np.asarray(WQ_w, np.float32),
        np.asarray(WQ_b, np.float32),
        np.asarray(WK_w, np.float32),
        np.asarray(WK_b, np.float32),
        np.asarray(WV_w, np.float32),
        WV_b,
    )
    res = _run(in_maps, trace=False)
    out = np.empty((B, S, D), dtype=np.float32)
    for core in range(N_CORES):
        b, half = core // 2, core % 2
        raw = res.results[core]["out"]  # [128, 16, CPC], seq = 128*t + p
        shard = raw.transpose(1, 0, 2).reshape(S, CPC).astype(np.float32)
        out[b, :, half * CPC : (half + 1) * CPC] = shard
    out += WV_b[None, None, :]
    return out
